# revision 23
# baseline (speedup 1.0000x reference)
"""Trainium2 Bass kernel for BestOfKSoftminOT (vector-form Sinkhorn, split-bf16
matmuls; ~452us vs 4042us baseline, rel err 2.9e-3 vs 2e-2 tolerance).

Math per (b, k) pair:
  X = sim_seq[b] [T,d]; Y = expert[b, s:s+T] [T,d]
  M = C/eps, C[i,j] = |x_i|^2 + |y_j|^2 - 2 x_i.y_j
  The reference runs 60 log-domain Sinkhorn iterations; the loss converges
  to ~2e-4 rel by ~15 effective iterations, so we run 1 exact log iteration
  + NFAST multiplicative vector iterations on a frozen plan P0.

All big matmuls use hi/lo-split bf16 operands (A ~ Ah+Al, B ~ Bh+Bl;
Sum A.B ~ Ah.Bh + Ah.Bl + Al.Bh as one 105-row bf16 contraction): fp32-class
accuracy (~2^-16) at 1 cycle/row streaming (fp32 matmuls are 4 cyc/row).
Operand row layout ([105, 512], blocks of 34):
  xa: [XAh; XAh; XAl; 1; 1; 0]    ya: [YAh; YAl; YAh; gv_h*; gv_l*; 0]
  xb: [XBh; XBh; XBl; gu_h*; gu_l*; 0]  yb: [YBh; YBl; YBh; 1; 1; 0]
(* = zero from host, written on-device; the final pass slices rows 0:102 to
exclude the gu rows.)

Device (per core; 16 pairs, 4 groups of 4):
  Warmup (exact, log-domain, per pair):
    row pass: mm = -M; gu = rowmin(M) - ln(T*sum_j exp(-(M-rowmin)))
    gu split hi/lo, PE-transposed, DMA'd into xb rows 102/103, so the col
    pass emits -M^T + gu directly; its stabilized exp is kept as P0T (bf16)
    and rescaled in place by 1/(T*sv) (e-trick).  gv likewise into ya;
    P0 = exp(-M + gv + gu_bias) (bf16); its accum seeds s_u (dv=1 row
    update for free).
  Fast loop (vector form; P0/P0T never rewritten):
    s_v[1,512] = sum_i P0[i,:] du_i  -- 16 bf16 matvecs per group, 4-way
    col-tiled into one PSUM bank (out partitions 0/32/64/96); ACT copy *T
    -> bf16; PE transpose; DVE strided evac + reciprocal_approx_fast -> dv.
    Symmetric row half against P0T.
  Final: Lk = eps * sum_ij du_i P0_ij dv_j M_ij via
    w2 = (-M^T)*dv_j*P0T (DVE stt over a streamed M^T recompute),
    z = ones^T @ w2, lk = accum(z * du_fm), du_fm = ACT-recip of T*s_u.
Host: builds split operands, softmin-over-K mean in fp64.
"""

import sys
from contextlib import ExitStack

import numpy as np

sys.path.insert(0, "/opt/trn_rl_repo")

import concourse.bass as bass
import concourse.bacc as bacc
import concourse.tile as tile
from concourse import mybir
from concourse.masks import make_identity
from concourse.bass_utils import run_bass_kernel_spmd

B, T, K, D = 16, 512, 8, 32
EPS, TAU = 0.1, 0.5
NCORES = 8
PAIRS = B * K // NCORES  # 16 pairs per core
NT = T // 128  # 4 chunks
NG = 4  # groups of 4 pairs
GSZ = 4
NFAST = 10  # multiplicative iterations; 1 warmup + NFAST total effective
NR = 105  # split-operand contraction rows (3 blocks of 34 + gu/gv rows)
F32 = mybir.dt.float32
BF16 = mybir.dt.bfloat16
ALU = mybir.AluOpType
AF = mybir.ActivationFunctionType


def _patch_act_tables():
    """Force activations into one table set so walrus doesn't thrash table
    loads between Exp/Ln/Reciprocal."""
    from concourse.hw_specs import get_activation_tables as real_gat

    keep = {"natural_log_exp_and_others", "reciprocal_and_small"}

    def patched(arch):
        tabs = real_gat(arch)
        return {
            name: (funcs if name in keep else set())
            for name, funcs in tabs.items()
        }

    bacc.get_activation_tables = patched


def _act_reciprocal(nc, out, in_):
    """ACT spline reciprocal; bass.activation() refuses Reciprocal for
    accuracy reasons, but ~1e-3 relative error is irrelevant here (verified
    against the reference numerically)."""
    eng = nc.scalar
    ins = [
        eng.lower_ap(in_),
        mybir.ImmediateValue(dtype=F32, value=0.0),
        mybir.ImmediateValue(dtype=F32, value=1.0),
        mybir.ImmediateValue(dtype=F32, value=0.0),
    ]
    return eng.add_instruction(
        mybir.InstActivation(
            name=nc.get_next_instruction_name(),
            func=AF.Reciprocal,
            ins=ins,
            outs=[eng.lower_ap(out)],
        )
    )


def build_program(pairs=PAIRS, nfast=NFAST):
    _patch_act_tables()
    nc = bacc.Bacc("TRN2")
    xa_d = nc.declare_dram_parameter("xa", [pairs, NR, 512], BF16, isOutput=False)
    ya_d = nc.declare_dram_parameter("ya", [pairs, NR, 512], BF16, isOutput=False)
    xb_d = nc.declare_dram_parameter("xb", [pairs, NR, 512], BF16, isOutput=False)
    yb_d = nc.declare_dram_parameter("yb", [pairs, NR, 512], BF16, isOutput=False)
    out_d = nc.declare_dram_parameter("out", [pairs, 1], F32, isOutput=True)

    with tile.TileContext(nc) as tc, ExitStack() as ctx:
        consts = ctx.enter_context(tc.tile_pool(name="consts", bufs=1))
        inp_ab = ctx.enter_context(tc.tile_pool(name="inpab", bufs=3))
        inp_st = ctx.enter_context(tc.tile_pool(name="inpst", bufs=pairs))
        pmat = ctx.enter_context(tc.tile_pool(name="pmat", bufs=pairs))
        small = ctx.enter_context(tc.tile_pool(name="small", bufs=pairs))
        g4p = ctx.enter_context(tc.tile_pool(name="g4p", bufs=4))
        grp = ctx.enter_context(tc.tile_pool(name="grp", bufs=NG))
        scr = ctx.enter_context(tc.tile_pool(name="scr", bufs=2))
        w2p = ctx.enter_context(tc.tile_pool(name="w2p", bufs=3))
        ps_s = ctx.enter_context(tc.tile_pool(name="pss", bufs=1, space="PSUM"))
        ps_tr = ctx.enter_context(tc.tile_pool(name="pstr", bufs=1, space="PSUM"))
        ps_mm = ctx.enter_context(tc.tile_pool(name="psmm", bufs=1, space="PSUM"))

        identb = consts.tile([128, 128], BF16)
        make_identity(nc, identb)
        onesb = consts.tile([128, 1], BF16)
        nc.vector.memset(onesb, 1.0)

        s_psum = [ps_s.tile([128, 512], F32, tag=f"s{g}", name=f"s{g}") for g in range(NG)]
        tr_psum = [ps_tr.tile([128, NT, 256], BF16, tag=f"tr{k}", name=f"tr{k}") for k in range(2)]
        mm_psum = [ps_mm.tile([128, 512], F32, tag=f"mm{k}", name=f"mm{k}") for k in range(2)]
        susb = [grp.tile([128, 512], BF16, tag="susb", name="susb") for _ in range(NG)]
        svsb = [grp.tile([128, 512], BF16, tag="svsb", name="svsb") for _ in range(NG)]
        stage = [grp.tile([128, NT, GSZ], F32, tag="stage", name="stage") for _ in range(NG)]
        recf = [grp.tile([128, NT, GSZ], F32, tag="recf", name="recf") for _ in range(NG)]
        du_all = [grp.tile([128, NT, GSZ], BF16, tag="du", name="du") for _ in range(NG)]
        csc_all = [grp.tile([128, NT, GSZ], F32, tag="csc", name="csc") for _ in range(NG)]
        dv_all = [grp.tile([128, NT, GSZ], BF16, tag="dv", name="dv") for _ in range(NG)]

        P0 = [pmat.tile([128, NT, 512], BF16, tag="P0", name="P0") for _ in range(pairs)]
        P0T = [pmat.tile([128, NT, 512], BF16, tag="P0T", name="P0T") for _ in range(pairs)]

        xb_t = [inp_st.tile([NR, 512], BF16, tag="xb", name="xbt") for _ in range(pairs)]
        yb_t = [inp_st.tile([NR, 512], BF16, tag="yb", name="ybt") for _ in range(pairs)]

        def transpose_hl(src_f32, dst_rows, ps_slot, hl_bf, hl4):
            """Split src [128, NT] f32 into hi/lo bf16, transpose to free-major,
            DMA the 8 chunk-rows into dst rows 102 (hi) / 103 (lo)."""
            nc.vector.tensor_copy(hl_bf[:, 0:NT], src_f32[:, :])  # hi (cast)
            nc.vector.tensor_sub(hl_bf[:, NT : 2 * NT], src_f32[:, :], hl_bf[:, 0:NT])
            nc.tensor.transpose(ps_slot[0:8, 0:128], hl_bf[:, :], identb[:, :])
            nc.vector.tensor_copy(hl4[:, :], ps_slot[0:8, 0:128])
            nc.gpsimd.dma_start(out=dst_rows[102:104, :], in_=hl4[:, :])

        # ---------------- Phase A: setup + exact warmup + materialize -------
        # Software-pipelined across pairs (3 stages) so engine streams
        # interleave independent pairs and fill each other's chain stalls.
        st = {}

        def stage1(p):
            g = p // GSZ
            xb2, yb2 = xb_t[p], yb_t[p]
            xa2 = inp_ab.tile([NR, 512], BF16, tag="xa", name="xa2")
            ya2 = inp_ab.tile([NR, 512], BF16, tag="ya", name="ya2")
            nc.sync.dma_start(out=xa2[:, :], in_=xa_d[p])
            nc.sync.dma_start(out=ya2[:, :], in_=ya_d[p])
            nc.sync.dma_start(out=xb2[:, :], in_=xb_d[p])
            nc.sync.dma_start(out=yb2[:, :], in_=yb_d[p])
            d = dict(xa2=xa2, ya2=ya2)
            for nm in ["gu", "ncm", "sv0", "lnv", "gv", "tsv", "su1", "tsu"]:
                d[nm] = small.tile([128, NT], F32, tag=nm, name=nm)
            d["pduf"] = small.tile([128, NT, 1], F32, tag="pduf", name="pduf")
            d["hlu"] = small.tile([128, 2 * NT], BF16, tag="hlu", name="hlu")
            d["hlv"] = small.tile([128, 2 * NT], BF16, tag="hlv", name="hlv")
            d["gu4"] = g4p.tile([8, 128], BF16, tag="gu4", name="gu4")
            d["gv4"] = g4p.tile([8, 128], BF16, tag="gv4", name="gv4")
            st[p] = d
            slots = [mm_psum[0], mm_psum[1], s_psum[g]]
            # row pass: gu = rowmin(M).  The -ln(T*sum exp) lse term is a pure
            # per-row rescaling of P0, absorbed exactly by the multiplicative
            # iteration (Sinkhorn scaling invariance), so it is skipped; the
            # col pass's exact stabilization keeps all entries <= 1/T.
            for t in range(NT):
                mm = slots[t % 3]
                nc.tensor.matmul(
                    mm[:, :],
                    xa2[:, t * 128 : (t + 1) * 128],
                    ya2[:, :],
                )
                nc.vector.tensor_reduce(
                    out=d["gu"][:, t : t + 1], in_=mm[:, :],
                    axis=mybir.AxisListType.X, op=ALU.max, negate=True,
                )
            transpose_hl(d["gu"], xb2, mm_psum[0].bitcast(BF16), d["hlu"], d["gu4"])

        def stage2(p):
            g, gi = p // GSZ, p % GSZ
            d = st[p]
            xb2, yb2 = xb_t[p], yb_t[p]
            slots = [mm_psum[0], mm_psum[1], s_psum[g]]
            # col pass on -M^T + gu; e-trick leaves P0T (unscaled) in place
            for t in range(NT):
                mm = slots[t % 3]
                nc.tensor.matmul(
                    mm[:, :],
                    yb2[:, t * 128 : (t + 1) * 128],
                    xb2[:, :],
                )
                nc.vector.tensor_reduce(
                    out=d["ncm"][:, t : t + 1], in_=mm[:, :],
                    axis=mybir.AxisListType.X, op=ALU.max, negate=True,
                )
                nc.scalar.activation(
                    P0T[p][:, t, :], mm[:, :], AF.Exp,
                    bias=d["ncm"][:, t : t + 1], scale=1.0,
                    accum_out=d["sv0"][:, t : t + 1],
                )
            nc.scalar.activation(d["lnv"][:, :], d["sv0"][:, :], AF.Ln, scale=float(T))
            nc.vector.tensor_sub(d["gv"][:, :], d["ncm"][:, :], d["lnv"][:, :])
            nc.vector.tensor_scalar_mul(d["tsv"][:, :], d["sv0"][:, :], float(T))
            nc.vector.reciprocal_approx_fast(
                out=csc_all[g][:, :, gi : gi + 1], in_=d["tsv"][:, :]
            )
            transpose_hl(d["gv"], d["ya2"], mm_psum[1].bitcast(BF16), d["hlv"], d["gv4"])

        def stage3(p):
            g, gi = p // GSZ, p % GSZ
            d = st.pop(p)
            slots = [mm_psum[0], mm_psum[1], s_psum[g]]
            # materialize P0 = exp(-M + gv + gu); accum seeds s_u (dv = 1)
            for t in range(NT):
                mm = slots[t % 3]
                nc.tensor.matmul(
                    mm[:, :],
                    d["xa2"][:, t * 128 : (t + 1) * 128],
                    d["ya2"][:, :],
                )
                nc.scalar.activation(
                    P0[p][:, t, :], mm[:, :], AF.Exp,
                    bias=d["gu"][:, t : t + 1], scale=1.0,
                    accum_out=d["su1"][:, t : t + 1],
                )
            nc.vector.tensor_scalar_mul(d["tsu"][:, :], d["su1"][:, :], float(T))
            nc.vector.reciprocal_approx_fast(out=d["pduf"][:, :, 0:1], in_=d["tsu"][:, :])
            nc.vector.tensor_copy(du_all[g][:, :, gi : gi + 1], d["pduf"][:, :, :])

        for p in range(pairs + 2):
            if p < pairs:
                stage1(p)
            if 1 <= p and p - 1 < pairs:
                stage2(p - 1)
            if 2 <= p:
                stage3(p - 2)

        # ---------------- Phase B: vector-form fast loop --------------------
        def half(rhs_mats, dvec, ssb, dst, fold=None):
            # matvecs: t-outer, (g, gi)-inner so PE always has independent
            # (bank, col-strip) streams in flight
            for t in range(NT):
                for g in range(NG):
                    for gi in range(GSZ):
                        p = GSZ * g + gi
                        nc.tensor.matmul(
                            s_psum[g][32 * gi : 32 * gi + 1, :],
                            dvec[g][:, t, gi : gi + 1],
                            rhs_mats[p][:, t, :],
                            start=(t == 0), stop=(t == NT - 1),
                            tile_position=(0, 32 * gi),
                            skip_group_check=True,
                        )
            for g in range(NG):
                nc.scalar.activation(
                    ssb[g][:, :], s_psum[g][:, :], AF.Copy, scale=float(T)
                )
            for g in range(NG):
                for c in range(NT):
                    nc.tensor.transpose(
                        tr_psum[g // 2][:, c, 128 * (g % 2) : 128 * (g % 2) + 128],
                        ssb[g][:, c * 128 : (c + 1) * 128],
                        identb[:, :],
                    )
            for g in range(NG):
                nc.vector.tensor_copy(
                    stage[g][:, :, :],
                    tr_psum[g // 2][:, :, 128 * (g % 2) : 128 * (g % 2) + 97 : 32],
                )
                nc.vector.reciprocal_approx_fast(out=recf[g][:, :, :], in_=stage[g][:, :, :])
                if fold is not None:
                    nc.vector.tensor_tensor(
                        out=recf[g][:, :, :], in0=recf[g][:, :, :],
                        in1=fold[g][:, :, :], op=ALU.mult,
                    )
                nc.vector.tensor_copy(dst[g][:, :, :], recf[g][:, :, :])

        for it in range(nfast):
            half(P0, du_all, svsb, dv_all, fold=csc_all)  # col update: dv*csc
            if it == nfast - 1:
                break
            half(P0T, dv_all, susb, du_all)  # row update: du

        # ---------------- Phase C: Lk = eps * sum du P0 dv M ----------------
        # susb holds T*s_u of the last row update; recf holds dv (f32).
        for g in range(NG):
            _act_reciprocal(nc, susb[g][:, :], susb[g][:, :])  # du free-major
            lkk = grp.tile([128, 1], F32, tag="lkk")
            for gi in range(GSZ):
                p = GSZ * g + gi
                xb2, yb2 = xb_t[p], yb_t[p]
                for t in range(NT):
                    mm = mm_psum[t % 2]
                    w2 = w2p.tile([128, 512], BF16, tag="w2")
                    nc.tensor.matmul(
                        mm[:, :],
                        yb2[0:102, t * 128 : (t + 1) * 128],
                        xb2[0:102, :],
                    )
                    if t % 2 == 0:
                        # ACT evac to bf16 puts the stt in 2x mode
                        mf = w2p.tile([128, 512], BF16, tag="mf")
                        nc.scalar.activation(mf[:, :], mm[:, :], AF.Copy)
                        src_ap = mf[:, :]
                    else:
                        src_ap = mm[:, :]  # direct PSUM read, 1x stt
                    nc.vector.scalar_tensor_tensor(
                        out=w2[:, :],
                        in0=src_ap,
                        scalar=recf[g][:, t, gi : gi + 1],
                        in1=P0T[p][:, t, :],
                        op0=ALU.mult,
                        op1=ALU.mult,
                    )
                    nc.tensor.matmul(
                        s_psum[g][32 * gi : 32 * gi + 1, :],
                        onesb[:, 0:1],
                        w2[:, :],
                        start=(t == 0), stop=(t == NT - 1),
                        tile_position=(0, 32 * gi),
                        skip_group_check=True,
                    )
            zs = scr.tile([128, 512], BF16, tag="escr")
            nc.vector.scalar_tensor_tensor(
                out=zs[:, :],
                in0=s_psum[g][:, :],
                scalar=1.0,
                in1=susb[g][:, :],
                op0=ALU.mult,
                op1=ALU.mult,
                accum_out=lkk[:, 0:1],
            )
            for gi in range(GSZ):
                p = GSZ * g + gi
                nc.sync.dma_start(out=out_d[p], in_=lkk[32 * gi : 32 * gi + 1, 0:1])

    nc.compile()
    return nc


def host_prep(sim_seq, expert, starts):
    """Build per-core hi/lo-split bf16 matmul operands [NR=105, 512]."""
    import ml_dtypes

    bf = ml_dtypes.bfloat16
    sim_seq = np.asarray(sim_seq, dtype=np.float32)
    expert = np.asarray(expert, dtype=np.float32)
    starts = np.asarray(starts).astype(np.int64)

    def split(a):
        h = a.astype(bf)
        l = (a - h.astype(np.float32)).astype(bf)
        return h, l

    in_maps = []
    for c in range(NCORES):
        xa = np.zeros((PAIRS, NR, 512), dtype=bf)
        ya = np.zeros((PAIRS, NR, 512), dtype=bf)
        xb = np.zeros((PAIRS, NR, 512), dtype=bf)
        yb = np.zeros((PAIRS, NR, 512), dtype=bf)
        for p in range(PAIRS):
            g = c * PAIRS + p
            b, k = g // K, g % K
            s = int(starts[b, k])
            X = sim_seq[b]  # [T, d]
            Y = expert[b, s : s + T]  # [T, d]
            xx = (X * X).sum(-1)
            yy = (Y * Y).sum(-1)
            XA = np.zeros((34, 512), dtype=np.float32)
            XA[:D] = X.T
            XA[D] = xx
            XA[D + 1] = 1.0
            YA = np.zeros((34, 512), dtype=np.float32)
            YA[:D] = (2.0 / EPS) * Y.T
            YA[D] = -1.0 / EPS
            YA[D + 1] = -yy / EPS
            XB = np.zeros((34, 512), dtype=np.float32)
            XB[:D] = (2.0 / EPS) * X.T
            XB[D] = -1.0 / EPS
            XB[D + 1] = -xx / EPS
            YB = np.zeros((34, 512), dtype=np.float32)
            YB[:D] = Y.T
            YB[D] = yy
            YB[D + 1] = 1.0
            XAh, XAl = split(XA)
            YAh, YAl = split(YA)
            XBh, XBl = split(XB)
            YBh, YBl = split(YB)
            # A-side: [h; h; l], B-side: [h; l; h]
            xa[p, 0:34], xa[p, 34:68], xa[p, 68:102] = XAh, XAh, XAl
            ya[p, 0:34], ya[p, 34:68], ya[p, 68:102] = YAh, YAl, YAh
            xb[p, 0:34], xb[p, 34:68], xb[p, 68:102] = XBh, XBh, XBl
            yb[p, 0:34], yb[p, 34:68], yb[p, 68:102] = YBh, YBl, YBh
            # potential-injection rows: lhs side carries 1s (exact in bf16)
            xa[p, 102] = 1.0
            xa[p, 103] = 1.0
            yb[p, 102] = 1.0
            yb[p, 103] = 1.0
        in_maps.append({"xa": xa, "ya": ya, "xb": xb, "yb": yb})
    return in_maps


def host_finish(results):
    Lk = np.zeros((B, K), dtype=np.float64)
    for c in range(NCORES):
        part = np.asarray(results[c]["out"], dtype=np.float64)  # [PAIRS, 1]
        for p in range(PAIRS):
            g = c * PAIRS + p
            Lk[g // K, g % K] = -EPS * part[p, 0]
    z = -Lk / TAU
    m = z.max(axis=1, keepdims=True)
    lse = m[:, 0] + np.log(np.exp(z - m).sum(axis=1))
    loss = -TAU * lse.mean()
    return np.float32(loss)


_CACHE = {}


def _get_program():
    if "nc" not in _CACHE:
        _CACHE["nc"] = build_program()
    return _CACHE["nc"]


def kernel(sim_seq, expert, starts):
    nc = _get_program()
    in_maps = host_prep(sim_seq, expert, starts)
    res = run_bass_kernel_spmd(nc, in_maps, list(range(NCORES)))
    return host_finish(res.results)


if __name__ == "__main__":
    import reference as ref

    inputs = ref.setup_inputs()
    expected = np.asarray(ref.reference(**inputs))
    actual = kernel(**{k: np.asarray(v) for k, v in inputs.items()})
    rel = abs(float(actual) - float(expected)) / abs(float(expected))
    print("expected:", expected, "actual:", actual, "rel err:", rel)


# revision 24
# speedup vs baseline: 1.1185x; 1.1185x over previous
"""Trainium2 Bass kernel for BestOfKSoftminOT (vector-form Sinkhorn, split-bf16
matmuls; ~452us vs 4042us baseline, rel err 2.9e-3 vs 2e-2 tolerance).

Math per (b, k) pair:
  X = sim_seq[b] [T,d]; Y = expert[b, s:s+T] [T,d]
  M = C/eps, C[i,j] = |x_i|^2 + |y_j|^2 - 2 x_i.y_j
  The reference runs 60 log-domain Sinkhorn iterations; the loss converges
  to ~2e-4 rel by ~15 effective iterations, so we run 1 exact log iteration
  + NFAST multiplicative vector iterations on a frozen plan P0.

All big matmuls use hi/lo-split bf16 operands (A ~ Ah+Al, B ~ Bh+Bl;
Sum A.B ~ Ah.Bh + Ah.Bl + Al.Bh as one 105-row bf16 contraction): fp32-class
accuracy (~2^-16) at 1 cycle/row streaming (fp32 matmuls are 4 cyc/row).
Operand row layout ([105, 512], blocks of 34):
  xa: [XAh; XAh; XAl; 1; 1; 0]    ya: [YAh; YAl; YAh; gv_h*; gv_l*; 0]
  xb: [XBh; XBh; XBl; gu_h*; gu_l*; 0]  yb: [YBh; YBl; YBh; 1; 1; 0]
(* = zero from host, written on-device; the final pass slices rows 0:102 to
exclude the gu rows.)

Device (per core; 16 pairs, 4 groups of 4):
  Warmup (exact, log-domain, per pair):
    row pass: mm = -M; gu = rowmin(M) - ln(T*sum_j exp(-(M-rowmin)))
    gu split hi/lo, PE-transposed, DMA'd into xb rows 102/103, so the col
    pass emits -M^T + gu directly; its stabilized exp is kept as P0T (bf16)
    and rescaled in place by 1/(T*sv) (e-trick).  gv likewise into ya;
    P0 = exp(-M + gv + gu_bias) (bf16); its accum seeds s_u (dv=1 row
    update for free).
  Fast loop (vector form; P0/P0T never rewritten):
    s_v[1,512] = sum_i P0[i,:] du_i  -- 16 bf16 matvecs per group, 4-way
    col-tiled into one PSUM bank (out partitions 0/32/64/96); ACT copy *T
    -> bf16; PE transpose; DVE strided evac + reciprocal_approx_fast -> dv.
    Symmetric row half against P0T.
  Final: Lk = eps * sum_ij du_i P0_ij dv_j M_ij via
    w2 = (-M^T)*dv_j*P0T (DVE stt over a streamed M^T recompute),
    z = ones^T @ w2, lk = accum(z * du_fm), du_fm = ACT-recip of T*s_u.
Host: builds split operands, softmin-over-K mean in fp64.
"""

import sys
from contextlib import ExitStack

import numpy as np

sys.path.insert(0, "/opt/trn_rl_repo")

import concourse.bass as bass
import concourse.bacc as bacc
import concourse.tile as tile
from concourse import mybir
from concourse.masks import make_identity
from concourse.bass_utils import run_bass_kernel_spmd

B, T, K, D = 16, 512, 8, 32
EPS, TAU = 0.1, 0.5
NCORES = 8
PAIRS = B * K // NCORES  # 16 pairs per core
NT = T // 128  # 4 chunks
NG = 4  # groups of 4 pairs
GSZ = 4
NFAST = 10  # multiplicative iterations; 1 warmup + NFAST total effective
NR = 105  # split-operand contraction rows (3 blocks of 34 + gu/gv rows)
F32 = mybir.dt.float32
BF16 = mybir.dt.bfloat16
ALU = mybir.AluOpType
AF = mybir.ActivationFunctionType


def _patch_act_tables():
    """Force activations into one table set so walrus doesn't thrash table
    loads between Exp/Ln/Reciprocal."""
    from concourse.hw_specs import get_activation_tables as real_gat

    keep = {"natural_log_exp_and_others", "reciprocal_and_small"}

    def patched(arch):
        tabs = real_gat(arch)
        return {
            name: (funcs if name in keep else set())
            for name, funcs in tabs.items()
        }

    bacc.get_activation_tables = patched


def _act_reciprocal(nc, out, in_):
    """ACT spline reciprocal; bass.activation() refuses Reciprocal for
    accuracy reasons, but ~1e-3 relative error is irrelevant here (verified
    against the reference numerically)."""
    eng = nc.scalar
    ins = [
        eng.lower_ap(in_),
        mybir.ImmediateValue(dtype=F32, value=0.0),
        mybir.ImmediateValue(dtype=F32, value=1.0),
        mybir.ImmediateValue(dtype=F32, value=0.0),
    ]
    return eng.add_instruction(
        mybir.InstActivation(
            name=nc.get_next_instruction_name(),
            func=AF.Reciprocal,
            ins=ins,
            outs=[eng.lower_ap(out)],
        )
    )


def build_program(pairs=PAIRS, nfast=NFAST):
    _patch_act_tables()
    nc = bacc.Bacc("TRN2")
    xa_d = nc.declare_dram_parameter("xa", [pairs, NR, 512], BF16, isOutput=False)
    ya_d = nc.declare_dram_parameter("ya", [pairs, NR, 512], BF16, isOutput=False)
    xb_d = nc.declare_dram_parameter("xb", [pairs, NR, 512], BF16, isOutput=False)
    yb_d = nc.declare_dram_parameter("yb", [pairs, NR, 512], BF16, isOutput=False)
    out_d = nc.declare_dram_parameter("out", [pairs, 1], F32, isOutput=True)

    with tile.TileContext(nc) as tc, ExitStack() as ctx:
        consts = ctx.enter_context(tc.tile_pool(name="consts", bufs=1))
        inp_ab = ctx.enter_context(tc.tile_pool(name="inpab", bufs=4))
        inp_st = ctx.enter_context(tc.tile_pool(name="inpst", bufs=pairs))
        pmat = ctx.enter_context(tc.tile_pool(name="pmat", bufs=pairs))
        small = ctx.enter_context(tc.tile_pool(name="small", bufs=pairs))
        g4p = ctx.enter_context(tc.tile_pool(name="g4p", bufs=4))
        grp = ctx.enter_context(tc.tile_pool(name="grp", bufs=NG))
        scr = ctx.enter_context(tc.tile_pool(name="scr", bufs=4))
        w2p = ctx.enter_context(tc.tile_pool(name="w2p", bufs=6))
        ps_s = ctx.enter_context(tc.tile_pool(name="pss", bufs=1, space="PSUM"))
        ps_tr = ctx.enter_context(tc.tile_pool(name="pstr", bufs=1, space="PSUM"))
        ps_mm = ctx.enter_context(tc.tile_pool(name="psmm", bufs=1, space="PSUM"))

        identb = consts.tile([128, 128], BF16)
        make_identity(nc, identb)
        onesb = consts.tile([128, 1], BF16)
        nc.vector.memset(onesb, 1.0)

        s_psum = [ps_s.tile([128, 512], F32, tag=f"s{g}", name=f"s{g}") for g in range(NG)]
        tr_psum = [ps_tr.tile([128, NT, 256], BF16, tag=f"tr{k}", name=f"tr{k}") for k in range(2)]
        mm_psum = [ps_mm.tile([128, 512], F32, tag=f"mm{k}", name=f"mm{k}") for k in range(2)]
        susb = [grp.tile([128, 512], BF16, tag="susb", name="susb") for _ in range(NG)]
        svsb = [grp.tile([128, 512], BF16, tag="svsb", name="svsb") for _ in range(NG)]
        stage = [grp.tile([128, NT, GSZ], F32, tag="stage", name="stage") for _ in range(NG)]
        recf = [grp.tile([128, NT, GSZ], F32, tag="recf", name="recf") for _ in range(NG)]
        du_all = [grp.tile([128, NT, GSZ], BF16, tag="du", name="du") for _ in range(NG)]
        csc_all = [grp.tile([128, NT, GSZ], F32, tag="csc", name="csc") for _ in range(NG)]
        dv_all = [grp.tile([128, NT, GSZ], BF16, tag="dv", name="dv") for _ in range(NG)]

        P0 = [pmat.tile([128, NT, 512], BF16, tag="P0", name="P0") for _ in range(pairs)]
        P0T = [pmat.tile([128, NT, 512], BF16, tag="P0T", name="P0T") for _ in range(pairs)]

        xb_t = [inp_st.tile([NR, 512], BF16, tag="xb", name="xbt") for _ in range(pairs)]
        yb_t = [inp_st.tile([NR, 512], BF16, tag="yb", name="ybt") for _ in range(pairs)]

        def transpose_hl(src_f32, dst_rows, ps_slot, hl_bf, hl4):
            """Split src [128, NT] f32 into hi/lo bf16, transpose to free-major,
            DMA the 8 chunk-rows into dst rows 102 (hi) / 103 (lo)."""
            nc.vector.tensor_copy(hl_bf[:, 0:NT], src_f32[:, :])  # hi (cast)
            nc.vector.tensor_sub(hl_bf[:, NT : 2 * NT], src_f32[:, :], hl_bf[:, 0:NT])
            nc.tensor.transpose(ps_slot[0:8, 0:128], hl_bf[:, :], identb[:, :])
            nc.vector.tensor_copy(hl4[:, :], ps_slot[0:8, 0:128])
            nc.gpsimd.dma_start(out=dst_rows[102:104, :], in_=hl4[:, :])

        # ---------------- Phase A: setup + exact warmup + materialize -------
        # Software-pipelined across pairs (3 stages) so engine streams
        # interleave independent pairs and fill each other's chain stalls.
        st = {}

        def stage1(p):
            g = p // GSZ
            xb2, yb2 = xb_t[p], yb_t[p]
            xa2 = inp_ab.tile([NR, 512], BF16, tag="xa", name="xa2")
            ya2 = inp_ab.tile([NR, 512], BF16, tag="ya", name="ya2")
            nc.sync.dma_start(out=xa2[:, :], in_=xa_d[p])
            nc.sync.dma_start(out=ya2[:, :], in_=ya_d[p])
            nc.sync.dma_start(out=xb2[:, :], in_=xb_d[p])
            nc.sync.dma_start(out=yb2[:, :], in_=yb_d[p])
            d = dict(xa2=xa2, ya2=ya2)
            for nm in ["gu", "ncm", "sv0", "lnv", "gv", "tsv", "su1", "tsu"]:
                d[nm] = small.tile([128, NT], F32, tag=nm, name=nm)
            d["pduf"] = small.tile([128, NT, 1], F32, tag="pduf", name="pduf")
            d["hlu"] = small.tile([128, 2 * NT], BF16, tag="hlu", name="hlu")
            d["hlv"] = small.tile([128, 2 * NT], BF16, tag="hlv", name="hlv")
            d["gu4"] = g4p.tile([8, 128], BF16, tag="gu4", name="gu4")
            d["gv4"] = g4p.tile([8, 128], BF16, tag="gv4", name="gv4")
            st[p] = d
            slots = [mm_psum[0], mm_psum[1], s_psum[g]]
            # row pass: gu = rowmin(M).  The -ln(T*sum exp) lse term is a pure
            # per-row rescaling of P0, absorbed exactly by the multiplicative
            # iteration (Sinkhorn scaling invariance), so it is skipped; the
            # col pass's exact stabilization keeps all entries <= 1/T.
            for t in range(NT):
                mm = slots[t % 3]
                nc.tensor.matmul(
                    mm[:, :],
                    xa2[:, t * 128 : (t + 1) * 128],
                    ya2[:, :],
                )
                nc.vector.tensor_reduce(
                    out=d["gu"][:, t : t + 1], in_=mm[:, :],
                    axis=mybir.AxisListType.X, op=ALU.max, negate=True,
                )
            transpose_hl(d["gu"], xb2, mm_psum[0].bitcast(BF16), d["hlu"], d["gu4"])

        def stage2(p):
            g, gi = p // GSZ, p % GSZ
            d = st[p]
            xb2, yb2 = xb_t[p], yb_t[p]
            slots = [mm_psum[0], mm_psum[1], s_psum[g]]
            # col pass on -M^T + gu; e-trick leaves P0T (unscaled) in place
            for t in range(NT):
                mm = slots[t % 3]
                nc.tensor.matmul(
                    mm[:, :],
                    yb2[:, t * 128 : (t + 1) * 128],
                    xb2[:, :],
                )
                nc.vector.tensor_reduce(
                    out=d["ncm"][:, t : t + 1], in_=mm[:, :],
                    axis=mybir.AxisListType.X, op=ALU.max, negate=True,
                )
                nc.scalar.activation(
                    P0T[p][:, t, :], mm[:, :], AF.Exp,
                    bias=d["ncm"][:, t : t + 1], scale=1.0,
                    accum_out=d["sv0"][:, t : t + 1],
                )
            nc.scalar.activation(d["lnv"][:, :], d["sv0"][:, :], AF.Ln, scale=float(T))
            nc.vector.tensor_sub(d["gv"][:, :], d["ncm"][:, :], d["lnv"][:, :])
            nc.vector.tensor_scalar_mul(d["tsv"][:, :], d["sv0"][:, :], float(T))
            nc.vector.reciprocal_approx_fast(
                out=csc_all[g][:, :, gi : gi + 1], in_=d["tsv"][:, :]
            )
            transpose_hl(d["gv"], d["ya2"], mm_psum[1].bitcast(BF16), d["hlv"], d["gv4"])

        def stage3(p):
            g, gi = p // GSZ, p % GSZ
            d = st.pop(p)
            slots = [mm_psum[0], mm_psum[1], s_psum[g]]
            # materialize P0 = exp(-M + gv + gu); accum seeds s_u (dv = 1)
            for t in range(NT):
                mm = slots[t % 3]
                nc.tensor.matmul(
                    mm[:, :],
                    d["xa2"][:, t * 128 : (t + 1) * 128],
                    d["ya2"][:, :],
                )
                nc.scalar.activation(
                    P0[p][:, t, :], mm[:, :], AF.Exp,
                    bias=d["gu"][:, t : t + 1], scale=1.0,
                    accum_out=d["su1"][:, t : t + 1],
                )
            nc.vector.tensor_scalar_mul(d["tsu"][:, :], d["su1"][:, :], float(T))
            nc.vector.reciprocal_approx_fast(out=d["pduf"][:, :, 0:1], in_=d["tsu"][:, :])
            nc.vector.tensor_copy(du_all[g][:, :, gi : gi + 1], d["pduf"][:, :, :])

        for p in range(pairs + 2):
            if p < pairs:
                stage1(p)
            if 1 <= p and p - 1 < pairs:
                stage2(p - 1)
            if 2 <= p:
                stage3(p - 2)

        # ---------------- Phase B: vector-form fast loop --------------------
        def half(rhs_mats, dvec, ssb, dst, fold=None):
            # matvecs: t-outer, (g, gi)-inner so PE always has independent
            # (bank, col-strip) streams in flight
            for t in range(NT):
                for g in range(NG):
                    for gi in range(GSZ):
                        p = GSZ * g + gi
                        nc.tensor.matmul(
                            s_psum[g][32 * gi : 32 * gi + 1, :],
                            dvec[g][:, t, gi : gi + 1],
                            rhs_mats[p][:, t, :],
                            start=(t == 0), stop=(t == NT - 1),
                            tile_position=(0, 32 * gi),
                            skip_group_check=True,
                        )
            for g in range(NG):
                nc.scalar.activation(
                    ssb[g][:, :], s_psum[g][:, :], AF.Copy, scale=float(T)
                )
            for g in range(NG):
                for c in range(NT):
                    nc.tensor.transpose(
                        tr_psum[g // 2][:, c, 128 * (g % 2) : 128 * (g % 2) + 128],
                        ssb[g][:, c * 128 : (c + 1) * 128],
                        identb[:, :],
                    )
            for g in range(NG):
                nc.vector.tensor_copy(
                    stage[g][:, :, :],
                    tr_psum[g // 2][:, :, 128 * (g % 2) : 128 * (g % 2) + 97 : 32],
                )
                nc.vector.reciprocal_approx_fast(out=recf[g][:, :, :], in_=stage[g][:, :, :])
                if fold is not None:
                    nc.vector.tensor_tensor(
                        out=recf[g][:, :, :], in0=recf[g][:, :, :],
                        in1=fold[g][:, :, :], op=ALU.mult,
                    )
                nc.vector.tensor_copy(dst[g][:, :, :], recf[g][:, :, :])

        for it in range(nfast):
            half(P0, du_all, svsb, dv_all, fold=csc_all)  # col update: dv*csc
            if it == nfast - 1:
                break
            half(P0T, dv_all, susb, du_all)  # row update: du

        # ---------------- Phase C: Lk = eps * sum du P0 dv M ----------------
        # susb holds T*s_u of the last row update; recf holds dv (f32).
        for g in range(NG):
            _act_reciprocal(nc, susb[g][:, :], susb[g][:, :])  # du free-major
            lkk = grp.tile([128, 1], F32, tag="lkk")
            for gi in range(GSZ):
                p = GSZ * g + gi
                xb2, yb2 = xb_t[p], yb_t[p]
                for t in range(NT):
                    mm = mm_psum[t % 2]
                    w2 = w2p.tile([128, 512], BF16, tag="w2")
                    mf = w2p.tile([128, 512], BF16, tag="mf")
                    nc.tensor.matmul(
                        mm[:, :],
                        yb2[0:102, t * 128 : (t + 1) * 128],
                        xb2[0:102, :],
                    )
                    nc.scalar.activation(mf[:, :], mm[:, :], AF.Copy)
                    nc.vector.scalar_tensor_tensor(
                        out=w2[:, :],
                        in0=mf[:, :],
                        scalar=recf[g][:, t, gi : gi + 1],
                        in1=P0T[p][:, t, :],
                        op0=ALU.mult,
                        op1=ALU.mult,
                    )
                    nc.tensor.matmul(
                        s_psum[g][32 * gi : 32 * gi + 1, :],
                        onesb[:, 0:1],
                        w2[:, :],
                        start=(t == 0), stop=(t == NT - 1),
                        tile_position=(0, 32 * gi),
                        skip_group_check=True,
                    )
            zs = scr.tile([128, 512], BF16, tag="escr")
            nc.vector.scalar_tensor_tensor(
                out=zs[:, :],
                in0=s_psum[g][:, :],
                scalar=1.0,
                in1=susb[g][:, :],
                op0=ALU.mult,
                op1=ALU.mult,
                accum_out=lkk[:, 0:1],
            )
            for gi in range(GSZ):
                p = GSZ * g + gi
                nc.sync.dma_start(out=out_d[p], in_=lkk[32 * gi : 32 * gi + 1, 0:1])

    nc.compile()
    return nc


def host_prep(sim_seq, expert, starts):
    """Build per-core hi/lo-split bf16 matmul operands [NR=105, 512]."""
    import ml_dtypes

    bf = ml_dtypes.bfloat16
    sim_seq = np.asarray(sim_seq, dtype=np.float32)
    expert = np.asarray(expert, dtype=np.float32)
    starts = np.asarray(starts).astype(np.int64)

    def split(a):
        h = a.astype(bf)
        l = (a - h.astype(np.float32)).astype(bf)
        return h, l

    in_maps = []
    for c in range(NCORES):
        xa = np.zeros((PAIRS, NR, 512), dtype=bf)
        ya = np.zeros((PAIRS, NR, 512), dtype=bf)
        xb = np.zeros((PAIRS, NR, 512), dtype=bf)
        yb = np.zeros((PAIRS, NR, 512), dtype=bf)
        for p in range(PAIRS):
            g = c * PAIRS + p
            b, k = g // K, g % K
            s = int(starts[b, k])
            X = sim_seq[b]  # [T, d]
            Y = expert[b, s : s + T]  # [T, d]
            xx = (X * X).sum(-1)
            yy = (Y * Y).sum(-1)
            XA = np.zeros((34, 512), dtype=np.float32)
            XA[:D] = X.T
            XA[D] = xx
            XA[D + 1] = 1.0
            YA = np.zeros((34, 512), dtype=np.float32)
            YA[:D] = (2.0 / EPS) * Y.T
            YA[D] = -1.0 / EPS
            YA[D + 1] = -yy / EPS
            XB = np.zeros((34, 512), dtype=np.float32)
            XB[:D] = (2.0 / EPS) * X.T
            XB[D] = -1.0 / EPS
            XB[D + 1] = -xx / EPS
            YB = np.zeros((34, 512), dtype=np.float32)
            YB[:D] = Y.T
            YB[D] = yy
            YB[D + 1] = 1.0
            XAh, XAl = split(XA)
            YAh, YAl = split(YA)
            XBh, XBl = split(XB)
            YBh, YBl = split(YB)
            # A-side: [h; h; l], B-side: [h; l; h]
            xa[p, 0:34], xa[p, 34:68], xa[p, 68:102] = XAh, XAh, XAl
            ya[p, 0:34], ya[p, 34:68], ya[p, 68:102] = YAh, YAl, YAh
            xb[p, 0:34], xb[p, 34:68], xb[p, 68:102] = XBh, XBh, XBl
            yb[p, 0:34], yb[p, 34:68], yb[p, 68:102] = YBh, YBl, YBh
            # potential-injection rows: lhs side carries 1s (exact in bf16)
            xa[p, 102] = 1.0
            xa[p, 103] = 1.0
            yb[p, 102] = 1.0
            yb[p, 103] = 1.0
        in_maps.append({"xa": xa, "ya": ya, "xb": xb, "yb": yb})
    return in_maps


def host_finish(results):
    Lk = np.zeros((B, K), dtype=np.float64)
    for c in range(NCORES):
        part = np.asarray(results[c]["out"], dtype=np.float64)  # [PAIRS, 1]
        for p in range(PAIRS):
            g = c * PAIRS + p
            Lk[g // K, g % K] = -EPS * part[p, 0]
    z = -Lk / TAU
    m = z.max(axis=1, keepdims=True)
    lse = m[:, 0] + np.log(np.exp(z - m).sum(axis=1))
    loss = -TAU * lse.mean()
    return np.float32(loss)


_CACHE = {}


def _get_program():
    if "nc" not in _CACHE:
        _CACHE["nc"] = build_program()
    return _CACHE["nc"]


def kernel(sim_seq, expert, starts):
    nc = _get_program()
    in_maps = host_prep(sim_seq, expert, starts)
    res = run_bass_kernel_spmd(nc, in_maps, list(range(NCORES)))
    return host_finish(res.results)


if __name__ == "__main__":
    import reference as ref

    inputs = ref.setup_inputs()
    expected = np.asarray(ref.reference(**inputs))
    actual = kernel(**{k: np.asarray(v) for k, v in inputs.items()})
    rel = abs(float(actual) - float(expected)) / abs(float(expected))
    print("expected:", expected, "actual:", actual, "rel err:", rel)
